# revision 31
# baseline (speedup 1.0000x reference)
# MoE top-2 routing kernel for 8 Trainium2 NeuronCores (expert-parallel),
# with a router-weight-ranked fp8 fast path. HW: 322.1-326.0 us over 6
# runs, median ~323 (was 371.7 us all-bf16; variance is startup DMA-ring
# phase luck in the fi=0 x-chase, visible as 0.4-2.5 us of early PE gaps).
# rel err 1.820e-2 vs the 2e-2 gate (deterministic, host-predicted
# exactly; host e4m3 emulation matched HW DoubleRow results to ~1e-4).
#
# Problem (hardcoded shapes): T=2048 tokens, D=2048 model dim, F=4096 ffn dim,
# E=8 experts, top-2 routing with renormalized softmax weights.
#
# Sharding: one expert per core; host does routing/dispatch. Per expert,
# the k2_e = load_e - C1 routed slots with the SMALLEST router weight run
# the whole MLP in fp8-e4m3 DoubleRow (one PE pass contracts 2 k-tiles =
# 2x throughput); the other C1 = roundup8(maxload - K2) slots stay bf16.
# Choosing k2_e per expert (not a flat count) makes the bf16 tier exactly
# full on every core and pushes all padding into the fp8 tier (zero cols,
# nearly free under the LDWEIGHTS roof), while keeping SPMD-uniform shapes.
# The output error contribution of an fp8 slot scales with its router
# weight w: full-fp8 error is 6.55% rel (measured host-side: e4m3 has only
# 3 mantissa bits; gate/up contribute 5.4%, down 3.8%), but the ranked
# slots carry a small share of the total w^2 mass. Error-vs-speed ladder
# (all host-verified, HW matches to 5 digits): K2=128/C1=408 -> 1.62e-2 at
# ~330 us; K2=144/C1=392 (CURRENT) -> 1.82e-2 at ~322 us; K2=160/C1=376 ->
# 2.01e-2 FAILS the gate. Useful C1 values sit where max(k2_e) lands on a
# multiple-of-16 shape: between-shapes C1 (e.g. 400/404) pays the next
# shape's fp8 MM time without the bf16 savings - measured net-worse.
#
# fp8 path details: weights pre-scaled by powers of 2 (wg x32, wu x8, wd
# x32) to clear the e4m3 subnormal floor (skipping this doubles weight
# quantization error), quantized host-side, laid out in
# DoubleRowSwInterleave order (pairs interleaved, columns reversed: LDW
# ~61 ns/pair vs ~75 plain DoubleRow - measured). The 1/32 gate scale
# folds into silu's scale input; the up/h scale (8) and down scale (32)
# fold into the router-weight broadcast (wb8 /= 256). h for fp8 tokens is
# produced directly in e4m3 by the DVE tensor_mul (absmax 8h ~ 99 << the
# 240 e4m3-TRN cap; beyond 240 converts to Inf -> NaN).
#
# PE economics (all HW-measured): a DoubleRow MM at N output cols costs
# N/2.4+2.5 ns like bf16 but contracts 2 k-tiles; its LDWEIGHTS costs ~61
# ns/pair (256 cols, no FWL), so fp8 sections are LDW-bound until the MM
# has ~140+ cols. At the K2=144 shape the fp8 MM (62.5 ns) outgrows its
# LDW entirely - pairs stream MM-bound at 62-63 ns with LDW fully hidden.
# Emission: fp8 sections GROUPED after the bf16 streams (interleaving fp8
# pairs into the bf16 stream costs ~15.5 ns extra on every following bf16
# MM - a bf16<->fp8 mode-switch cost paid per switch, 11.5 us total;
# grouped pays it ~2x per fi). Each fi's fp8 section is deferred into
# fi+1's body and its weight/x8 DMAs enqueued after fi+1's bf16 weights:
# the FIFO ring otherwise delays fi=1's startup-critical weights by the
# 0.73MB of fp8 bytes (was a 2.2-2.7 us PE gap at ~12 us).
# Stream accounting at C1=392: 1536 bf16 MMs x 165.8 + 768 fp8 pairs x
# 62.5 + 42 warmup x 109 + ~45 switches x 15 = 309.5 us busy, 0.4 us gaps,
# 7.1 us fixed runtime preamble, 5.1 us tail (last store chain ~2.5 us +
# completion barrier ~2.6 us; a split last chunk measured no better).
# Warmup 42 dummies is tuned: 24 measured +2.0 us (cold-clock exposure),
# 38 measured equal within the +-0.8 us run noise.
#
# Rejected with measurements: fp8-everything (2x PE, ~185 us, but 6.55%
# err fails the gate); threshold-based tiers (per-tier capacity = max over
# cores reintroduces padding; rank-based is pad-free); hi/lo residual
# splitting (xh@wh + xl@wh + xh@wl needs 3 DR column-passes vs 2
# bf16-equivalents - DR doubles contraction per pass, not column rate, so
# it is 1.5x SLOWER than bf16; one-sided splits of the LDW-bound sections
# double the fp8 MM time and erase the tier's entire win); expert
# pairing/F-splits (SPMD-uniform capacities eat the balance win: CA+CB >=
# 536+501); int8/uint8 matmul (not a TRN2 matmul dtype); e3m4 (4 mantissa
# bits but no DoubleRow -> 1x only); MX/x4 formats (trn3-only).
# fp8 DoublePixel/DoubleColumn are unexplored (cost model: "TODO").
#
# --- bf16 baseline architecture (unchanged) ---
# Device layout is fully weight-stationary, tokens always moving in columns:
#   g[f,t] = sum_d wg[d,f] x[d,t]   (lhsT = 128x128 wg tile, rhs = xT cols)
#   u[f,t] likewise; h[f,t] = silu(g)*u lands directly in [f,t] layout, so
#   the down matmul y[d,t] = sum_f wd[f,d] h[f,t] needs NO PE transposes.
# Router pre-pass on host picks top-2 (selection gap >> fp32 noise),
# computes renormalized weights in float64, gathers each expert's tokens
# into transposed capacity buffers, zero-pads tails (MLP(0)=0, wb=0).
# Weights host-retiled so every weight DMA is one [128, D|F] contiguous
# block. ~5us of throwaway matmuls at program start keep the PE busy while
# startup DMAs land so the HAM clock-gate opens to 8/8 before the stream.
# Startup DMA order on the sync HWDGE ring: fi=0 gate weights, first x
# group, fi=0 up weights, remaining x groups; fi=0 interleaves gate/up per
# 4-d-tile group so the PE chases the incoming x stream. Output is yT; the
# host scatter-adds transposes into [T, D].

import os
import numpy as np
import ml_dtypes

_BF16NP = ml_dtypes.bfloat16
_E4NP = ml_dtypes.float8_e4m3   # == TRN FP8_EXP4 (bias 7, max 240)

import concourse.bacc as bacc
import concourse.mybir as mybir
import concourse.tile as tile
from concourse import bass_utils

FP32 = mybir.dt.float32
BF16 = mybir.dt.bfloat16
FP8 = mybir.dt.float8e4
ACTF = mybir.ActivationFunctionType
PM = mybir.MatmulPerfMode

T, D, F, E = 2048, 2048, 4096, 8
NCORES = 8
ND = D // 128    # 16 d-tiles
NF = F // 128    # 32 f-tiles

K2 = 144         # fp8-tier capacity (multiple of 16); per-expert count is
                 # k2_e = load_e - C1 so the bf16 tier is exactly full for
                 # every expert and all padding lands in the fp8 tier, where
                 # zero columns ride (nearly) free under the LDWEIGHTS roof
SG = 32.0        # wg8 pre-scale (folded out via silu's scale input)
SU = 8.0         # wu8 pre-scale (rides through h8; folded into wb8)
SD = 32.0        # wd8 pre-scale (folded into wb8)
SWI = True       # DoubleRowSwInterleave weight layout


def _chunks_for(C):
    """Split C token columns into PSUM-bank-sized chunks (<=512 fp32 cols)."""
    nch = (C + 511) // 512
    out, rem, c0 = [], C, 0
    for i in range(nch):
        cn = -(-(rem // (nch - i)) // 4) * 4
        cn = min(cn, rem)
        out.append((c0, cn))
        c0 += cn
        rem -= cn
    return out


def build_program(C1):
    chunks = _chunks_for(C1)
    pm = PM.DoubleRowSwInterleave if SWI else PM.DoubleRow
    nc = bacc.Bacc(
        "TRN2",
        target_bir_lowering=False,
        debug=False,
        enable_asserts=False,
        num_devices=NCORES,
    )
    # bf16-tier x in [p, d, t] tile layout [128, ND*C1]
    x_d = nc.dram_tensor("x", [128, ND * C1], BF16, kind="ExternalInput").ap()
    # fp8-tier x, same layout with C=K2
    x8_d = nc.dram_tensor("x8", [128, ND * K2], FP8, kind="ExternalInput").ap()
    # router weight per token, broadcast to [128, C] on host, fp32
    wb_d = nc.dram_tensor("wb", [128, C1], FP32, kind="ExternalInput").ap()
    wb8_d = nc.dram_tensor("wb8", [128, K2], FP32, kind="ExternalInput").ap()
    # retiled weights: wg/wu rows fi*128+p, cols d*128+q (= wg[d*128+p, fi*128+q])
    wg_d = nc.dram_tensor("wg", [F, D], BF16, kind="ExternalInput").ap()
    wu_d = nc.dram_tensor("wu", [F, D], BF16, kind="ExternalInput").ap()
    # retiled wd: rows dt*128+p, cols fi*128+q  (= wd[fi*128+p, dt*128+q])
    wd_d = nc.dram_tensor("wd", [D, F], BF16, kind="ExternalInput").ap()
    # fp8 copies (pre-scaled, SwInterleaved pair layout)
    wg8_d = nc.dram_tensor("wg8", [F, D], FP8, kind="ExternalInput").ap()
    wu8_d = nc.dram_tensor("wu8", [F, D], FP8, kind="ExternalInput").ap()
    wd8_d = nc.dram_tensor("wd8", [D, F], FP8, kind="ExternalInput").ap()
    # outputs yT fp32, per tier
    y_d = nc.dram_tensor("y", [D, C1], FP32, kind="ExternalOutput").ap()
    y8_d = nc.dram_tensor("y8", [D, K2], FP32, kind="ExternalOutput").ap()

    with tile.TileContext(nc) as tc:
        with (
            tc.tile_pool(name="const", bufs=1) as const_pool,
            tc.tile_pool(name="xp", bufs=1) as x_pool,
            tc.tile_pool(name="hp", bufs=1) as h_pool,
            tc.tile_pool(name="wgu", bufs=8) as wgu_pool,
            tc.tile_pool(name="wgu8", bufs=8) as wgu8_pool,
            tc.tile_pool(name="wdp", bufs=3) as wd_pool,
            tc.tile_pool(name="wdp8", bufs=3) as wd8_pool,
            tc.tile_pool(name="yp", bufs=4) as y_pool,
            tc.tile_pool(name="stp", bufs=4) as st_pool,
            tc.tile_pool(name="ps", bufs=4, space="PSUM") as ps_pool,
            tc.tile_pool(name="ps8", bufs=4, space="PSUM") as ps8_pool,
        ):
            # ---- PE warmup: throwaway matmuls so the HAM clock-gate opens
            # to 8/8 while the startup DMAs land. ----
            dum = const_pool.tile([128, 160], BF16, tag="dum", name="dum")
            nc.vector.memset(dum[:], 0.0)
            pdum = ps_pool.tile([128, 512], FP32, tag="ps", name="ps")
            for _ in range(42):
                nc.tensor.matmul(pdum[:, :160], dum[:, :128], dum[:],
                                 start=True, stop=True)

            # startup-critical DMA order on the sync HWDGE ring (FIFO):
            # fi=0 bf16 gate weights, first x group, fi=0 bf16 up weights,
            # remaining x groups, fi=0 fp8 weights, fp8 x. wb deferred.
            wgt0 = wgu_pool.tile([128, D], BF16, tag="w", name="wgt")
            nc.sync.dma_start(wgt0[:], wg_d[0:128, :])
            xt = [x_pool.tile([128, 4 * C1], BF16, tag=f"x{g}", name=f"x{g}")
                  for g in range(ND // 4)]
            nc.sync.dma_start(xt[0][:], x_d[:, 0:4 * C1])
            wut0 = wgu_pool.tile([128, D], BF16, tag="w", name="wut")
            nc.sync.dma_start(wut0[:], wu_d[0:128, :])
            for g in range(1, ND // 4):
                nc.sync.dma_start(xt[g][:], x_d[:, g * 4 * C1:(g + 1) * 4 * C1])
            # fi=0 fp8 weights + x8 are enqueued inside fi=1's body, after
            # fi=1's startup-critical bf16 weights (the ring is FIFO and the
            # deferred fi=0 fp8 section doesn't run until ~18us in)
            wgt80 = wgu8_pool.tile([128, ND, 128], FP8, tag="w8", name="wgt8")
            wut80 = wgu8_pool.tile([128, ND, 128], FP8, tag="w8", name="wut8")
            x8t = x_pool.tile([128, ND, K2], FP8, tag="x8", name="x8t")

            def xs(d, c0, cn):
                return xt[d // 4][:, (d % 4) * C1 + c0:(d % 4) * C1 + c0 + cn]

            # ---- phase 1: gate/up matmuls + silu*up -> h[f, t] ----
            # The fp8 section for fi is emitted one fi later (inside fi+1's
            # body): at fi=0 this moves its weights/x8 out of the startup
            # DMA crunch; keeping the shift uniform keeps the stream simple.
            hs = h_pool.tile([128, NF, C1], BF16, tag="h", name="h")
            hs8 = h_pool.tile([128, NF, K2], FP8, tag="h8", name="h8")

            def fp8_gu_section(fi, wgt8, wut8):
                # grouped fp8 DR sections: at K2=128 the fp8 MM (~56 ns)
                # nearly covers its own LDW (~61 ns), and grouping pays the
                # bf16<->fp8 mode switch once per section instead of per pair
                pg8 = ps8_pool.tile([128, 512], FP32, tag="ps8", name="ps8")
                pu8 = ps8_pool.tile([128, 512], FP32, tag="ps8", name="ps8")
                for dp in range(ND // 2):
                    nc.tensor.matmul(
                        pg8[:, :K2], wgt8[:, 2 * dp:2 * dp + 2, :],
                        x8t[:, 2 * dp:2 * dp + 2, :],
                        start=(dp == 0), stop=(dp == ND // 2 - 1),
                        perf_mode=pm,
                    )
                for dp in range(ND // 2):
                    nc.tensor.matmul(
                        pu8[:, :K2], wut8[:, 2 * dp:2 * dp + 2, :],
                        x8t[:, 2 * dp:2 * dp + 2, :],
                        start=(dp == 0), stop=(dp == ND // 2 - 1),
                        perf_mode=pm,
                    )
                st8 = st_pool.tile([128, 512], FP32, tag="st", name="st")
                nc.scalar.activation(st8[:, :K2], pg8[:, :K2], ACTF.Silu,
                                     scale=1.0 / SG)
                nc.vector.tensor_mul(hs8[:, fi, :], st8[:, :K2], pu8[:, :K2])

            pend8 = None
            for fi in range(NF):
                if fi == 0:
                    wgt, wut, wgt8, wut8 = wgt0, wut0, wgt80, wut80
                else:
                    wgt = wgu_pool.tile([128, D], BF16, tag="w", name="wgt")
                    nc.sync.dma_start(wgt[:], wg_d[fi * 128:(fi + 1) * 128, :])
                    wut = wgu_pool.tile([128, D], BF16, tag="w", name="wut")
                    nc.sync.dma_start(wut[:], wu_d[fi * 128:(fi + 1) * 128, :])
                    if fi == 1:
                        nc.sync.dma_start(wgt80[:], wg8_d[0:128, :])
                        nc.sync.dma_start(wut80[:], wu8_d[0:128, :])
                        nc.sync.dma_start(x8t[:], x8_d[:])
                    wgt8 = wgu8_pool.tile([128, ND, 128], FP8, tag="w8",
                                          name="wgt8")
                    nc.sync.dma_start(wgt8[:], wg8_d[fi * 128:(fi + 1) * 128, :])
                    wut8 = wgu8_pool.tile([128, ND, 128], FP8, tag="w8",
                                          name="wut8")
                    nc.sync.dma_start(wut8[:], wu8_d[fi * 128:(fi + 1) * 128, :])
                pg = [ps_pool.tile([128, 512], FP32, tag="ps", name="ps")
                      for _ in chunks]
                pu = [ps_pool.tile([128, 512], FP32, tag="ps", name="ps")
                      for _ in chunks]
                if fi == 0:
                    # chase the incoming x stream per 4-d-tile group
                    for g in range(ND // 4):
                        for d in range(4 * g, 4 * g + 4):
                            lw = wgt[:, d * 128:(d + 1) * 128]
                            for ci, (c0, cn) in enumerate(chunks):
                                nc.tensor.matmul(
                                    pg[ci][:, :cn], lw, xs(d, c0, cn),
                                    start=(d == 0), stop=(d == ND - 1),
                                )
                        for d in range(4 * g, 4 * g + 4):
                            lw = wut[:, d * 128:(d + 1) * 128]
                            for ci, (c0, cn) in enumerate(chunks):
                                nc.tensor.matmul(
                                    pu[ci][:, :cn], lw, xs(d, c0, cn),
                                    start=(d == 0), stop=(d == ND - 1),
                                )
                else:
                    for d in range(ND):
                        lw = wgt[:, d * 128:(d + 1) * 128]
                        for ci, (c0, cn) in enumerate(chunks):
                            nc.tensor.matmul(
                                pg[ci][:, :cn], lw, xs(d, c0, cn),
                                start=(d == 0), stop=(d == ND - 1),
                            )
                    for d in range(ND):
                        lw = wut[:, d * 128:(d + 1) * 128]
                        for ci, (c0, cn) in enumerate(chunks):
                            nc.tensor.matmul(
                                pu[ci][:, :cn], lw, xs(d, c0, cn),
                                start=(d == 0), stop=(d == ND - 1),
                            )
                if pend8 is not None:
                    fp8_gu_section(*pend8)
                pend8 = (fi, wgt8, wut8)
                for ci, (c0, cn) in enumerate(chunks):
                    st = st_pool.tile([128, 512], FP32, tag="st", name="st")
                    nc.scalar.activation(st[:, :cn], pg[ci][:, :cn], ACTF.Silu)
                    nc.vector.tensor_mul(hs[:, fi, c0:c0 + cn], st[:, :cn],
                                         pu[ci][:, :cn])
            fp8_gu_section(*pend8)

            # wb first consumed by phase 2; keep out of the startup crunch
            wb_sb = const_pool.tile([128, C1], FP32, tag="wb", name="wb_sb")
            nc.sync.dma_start(wb_sb[:], wb_d[:])
            wb8_sb = const_pool.tile([128, K2], FP32, tag="wb8", name="wb8_sb")
            nc.sync.dma_start(wb8_sb[:], wb8_d[:])

            # ---- phase 2: down matmuls, router-weight scale, store yT ----
            for dt in range(ND):
                last = dt == ND - 1
                wdt = wd_pool.tile([128, F], BF16, tag="wd", name="wdt")
                wdt8 = wd8_pool.tile([128, NF, 128], FP8, tag="wd8",
                                     name="wdt8")
                if last:
                    # fp8 first for the last dt so its store isn't the tail
                    nc.sync.dma_start(wdt8[:], wd8_d[dt * 128:(dt + 1) * 128, :])
                    nc.sync.dma_start(wdt[:], wd_d[dt * 128:(dt + 1) * 128, :])
                else:
                    nc.sync.dma_start(wdt[:], wd_d[dt * 128:(dt + 1) * 128, :])
                    nc.sync.dma_start(wdt8[:], wd8_d[dt * 128:(dt + 1) * 128, :])

                def fp8_dn_section():
                    py8 = ps8_pool.tile([128, 512], FP32, tag="ps8",
                                        name="ps8")
                    for fp in range(NF // 2):
                        nc.tensor.matmul(
                            py8[:, :K2], wdt8[:, 2 * fp:2 * fp + 2, :],
                            hs8[:, 2 * fp:2 * fp + 2, :],
                            start=(fp == 0), stop=(fp == NF // 2 - 1),
                            perf_mode=pm,
                        )
                    y8sb = y_pool.tile([128, 512], FP32, tag="y", name="y8sb")
                    nc.vector.tensor_mul(y8sb[:, :K2], py8[:, :K2], wb8_sb[:])
                    nc.sync.dma_start(y8_d[dt * 128:(dt + 1) * 128, :],
                                      y8sb[:, :K2])

                if last:
                    fp8_dn_section()
                for ci, (c0, cn) in enumerate(chunks):
                    py = ps_pool.tile([128, 512], FP32, tag="ps", name="ps")
                    for fi in range(NF):
                        nc.tensor.matmul(
                            py[:, :cn], wdt[:, fi * 128:(fi + 1) * 128],
                            hs[:, fi, c0:c0 + cn],
                            start=(fi == 0), stop=(fi == NF - 1),
                        )
                    ysb = y_pool.tile([128, 512], FP32, tag="y", name="ysb")
                    nc.vector.tensor_mul(ysb[:, :cn], py[:, :cn],
                                         wb_sb[:, c0:c0 + cn])
                    nc.sync.dma_start(
                        y_d[dt * 128:(dt + 1) * 128, c0:c0 + cn], ysb[:, :cn])
                if not last:
                    fp8_dn_section()

    nc.compile()
    return nc


_PROGRAM_CACHE = {}


def _get_program(C1):
    if C1 not in _PROGRAM_CACHE:
        _PROGRAM_CACHE[C1] = build_program(C1)
    return _PROGRAM_CACHE[C1]


def _route_host(x_TD, router_w):
    """Host dispatch: top-2 ids + renormalized top-2 softmax weights."""
    logits = (x_TD @ router_w).astype(np.float64)  # selection gap >> fp32 err
    order = np.argsort(-logits, axis=1, kind="stable")
    top2 = order[:, :2]
    z = logits - logits.max(axis=1, keepdims=True)
    p = np.exp(z)
    p /= p.sum(axis=1, keepdims=True)
    pw = np.take_along_axis(p, top2, axis=1)       # [T, 2]
    pw /= pw.sum(axis=1, keepdims=True)
    return top2, pw


def _retile_wgu(w):
    """[D, F] -> [F, D] rows fi*128+p, cols d*128+q."""
    m = w.reshape(ND, 128, NF, 128).transpose(2, 1, 0, 3)
    return np.ascontiguousarray(m).reshape(F, D)


def _retile_wd(w):
    """[F, D] -> [D, F] rows dt*128+p, cols fi*128+q."""
    m = w.reshape(NF, 128, ND, 128).transpose(2, 1, 0, 3)
    return np.ascontiguousarray(m).reshape(D, F)


def _swinterleave(w):
    """[128, n, 128] retiled rows -> DoubleRowSwInterleave pair layout.

    Physical per-partition pair order: A127,B127,A126,B126,...,A0,B0."""
    p, n, m = w.shape
    a = w[:, 0::2, ::-1]    # [p, n/2, m] reversed cols
    b = w[:, 1::2, ::-1]
    il = np.empty((p, n // 2, 2 * m), w.dtype)
    il[:, :, 0::2] = a
    il[:, :, 1::2] = b
    return il.reshape(p, n, m)


def _fp8_weights(w, scale, down):
    """Quantize a [D,F]/[F,D] weight to scaled e4m3 in retiled+DR layout."""
    q = (w * scale).astype(_E4NP)
    r = _retile_wd(q) if down else _retile_wgu(q)   # [nrow, ncol]
    nrow, ncol = r.shape
    r3 = r.reshape(nrow // 128, 128, ncol // 128, 128)
    if SWI:
        r3 = np.stack([_swinterleave(blk) for blk in r3])
    return np.ascontiguousarray(r3.reshape(nrow, ncol))


def _xe_tile(xg, Ccap, dtype):
    """[n<=Ccap, D] tokens -> [128, ND*Ccap] [p, d, t] tile layout."""
    n = xg.shape[0]
    buf = np.zeros((Ccap, D), np.float32)
    buf[:n] = xg
    m = buf.T.astype(dtype).reshape(ND, 128, Ccap).transpose(1, 0, 2)
    return np.ascontiguousarray(m).reshape(128, ND * Ccap)


def kernel_with_results(x_TD, router_w, w_gate, w_up, w_down):
    x_TD = np.ascontiguousarray(x_TD, np.float32)
    router_w = np.ascontiguousarray(router_w, np.float32)
    w_gate = np.ascontiguousarray(w_gate, np.float32)
    w_up = np.ascontiguousarray(w_up, np.float32)
    w_down = np.ascontiguousarray(w_down, np.float32)

    top2, pw = _route_host(x_TD, router_w)
    # per expert: slot lists (token idx, weight), sorted by weight ascending
    slot_ix, slot_w = [], []
    for e in range(E):
        ixs, ws = [], []
        for s in range(2):
            ix = np.where(top2[:, s] == e)[0]
            ixs.append(ix)
            ws.append(pw[ix, s])
        allix = np.concatenate(ixs)
        allw = np.concatenate(ws)
        o = np.argsort(allw, kind="stable")
        slot_ix.append(allix[o])
        slot_w.append(allw[o])
    loads = [len(ix) for ix in slot_ix]
    C1 = max(64, -(-(max(loads) - K2) // 8) * 8)
    k2s = [max(0, min(K2, L - C1)) for L in loads]

    nc = _get_program(C1)

    in_maps = []
    for e in range(E):
        k2 = k2s[e]
        ix8 = slot_ix[e][:k2]
        w8 = slot_w[e][:k2]
        ix1 = slot_ix[e][k2:]
        w1 = slot_w[e][k2:]
        n1 = len(ix1)

        wtok = np.zeros((C1,), np.float32)
        wtok[:n1] = w1
        wb = np.ascontiguousarray(
            np.broadcast_to(wtok[None, :], (128, C1)), np.float32)
        wtok8 = np.zeros((K2,), np.float32)
        wtok8[:k2] = w8 / (SU * SD)
        wb8 = np.ascontiguousarray(
            np.broadcast_to(wtok8[None, :], (128, K2)), np.float32)

        x8buf = np.zeros((K2, D), np.float32)
        x8buf[:k2] = x_TD[ix8]

        in_maps.append({
            "x": _xe_tile(x_TD[ix1], C1, _BF16NP),
            "x8": _xe_tile(x8buf, K2, _E4NP),
            "wb": wb,
            "wb8": wb8,
            "wg": _retile_wgu(w_gate[e].astype(_BF16NP)),
            "wu": _retile_wgu(w_up[e].astype(_BF16NP)),
            "wd": _retile_wd(w_down[e].astype(_BF16NP)),
            "wg8": _fp8_weights(w_gate[e], SG, False),
            "wu8": _fp8_weights(w_up[e], SU, False),
            "wd8": _fp8_weights(w_down[e], SD, True),
        })

    try:
        res = bass_utils.run_bass_kernel_spmd(
            nc, in_maps, core_ids=list(range(NCORES))
        )
    except ModuleNotFoundError:
        os.environ["BASS_NEVER_TRACE"] = "1"
        res = bass_utils.run_bass_kernel_spmd(
            nc, in_maps, core_ids=list(range(NCORES))
        )

    out = np.zeros((T, D), np.float32)
    for e in range(E):
        k2 = k2s[e]
        ix8 = slot_ix[e][:k2]
        ix1 = slot_ix[e][k2:]
        y = res.results[e]["y"]    # [D, C1]
        y8 = res.results[e]["y8"]  # [D, K2]
        out[ix1] += y[:, :len(ix1)].T
        out[ix8] += y8[:, :len(ix8)].T
    return out, res


def kernel(**inputs):
    out, _ = kernel_with_results(**inputs)
    return out


# revision 32
# speedup vs baseline: 1.0033x; 1.0033x over previous
# MoE top-2 routing kernel for 8 Trainium2 NeuronCores (expert-parallel),
# with a router-weight-ranked fp8 fast path. HW: 322.1-326.4 us over 10
# runs, median 323.4 (was 371.7 us all-bf16; variance is startup DMA-ring
# phase luck in the fi=0 x-chase, visible as 0.4-2.5 us of early PE gaps).
# rel err 1.820e-2 vs the 2e-2 gate (deterministic, host-predicted
# exactly; host e4m3 emulation matched HW DoubleRow results to ~1e-4).
#
# Problem (hardcoded shapes): T=2048 tokens, D=2048 model dim, F=4096 ffn dim,
# E=8 experts, top-2 routing with renormalized softmax weights.
#
# Sharding: one expert per core; host does routing/dispatch. Per expert,
# the k2_e = load_e - C1 routed slots with the SMALLEST router weight run
# the whole MLP in fp8-e4m3 DoubleRow (one PE pass contracts 2 k-tiles =
# 2x throughput); the other C1 = roundup8(maxload - K2) slots stay bf16.
# Choosing k2_e per expert (not a flat count) makes the bf16 tier exactly
# full on every core and pushes all padding into the fp8 tier (zero cols,
# nearly free under the LDWEIGHTS roof), while keeping SPMD-uniform shapes.
# The output error contribution of an fp8 slot scales with its router
# weight w: full-fp8 error is 6.55% rel (measured host-side: e4m3 has only
# 3 mantissa bits; gate/up contribute 5.4%, down 3.8%), but the ranked
# slots carry a small share of the total w^2 mass. Error-vs-speed ladder
# (all host-verified, HW matches to 5 digits): K2=128/C1=408 -> 1.62e-2 at
# ~330 us; K2=144/C1=392 (CURRENT) -> 1.82e-2 at ~322 us; K2=160/C1=376 ->
# 2.01e-2 FAILS the gate. Useful C1 values sit where max(k2_e) lands on a
# multiple-of-16 shape: between-shapes C1 (e.g. 400/404) pays the next
# shape's fp8 MM time without the bf16 savings - measured net-worse.
#
# fp8 path details: weights pre-scaled by powers of 2 (wg x32, wu x8, wd
# x32) to clear the e4m3 subnormal floor (skipping this doubles weight
# quantization error), quantized host-side, laid out in
# DoubleRowSwInterleave order (pairs interleaved, columns reversed: LDW
# ~61 ns/pair vs ~75 plain DoubleRow - measured). The 1/32 gate scale
# folds into silu's scale input; the up/h scale (8) and down scale (32)
# fold into the router-weight broadcast (wb8 /= 256). h for fp8 tokens is
# produced directly in e4m3 by the DVE tensor_mul (absmax 8h ~ 99 << the
# 240 e4m3-TRN cap; beyond 240 converts to Inf -> NaN).
#
# PE economics (all HW-measured): a DoubleRow MM at N output cols costs
# N/2.4+2.5 ns like bf16 but contracts 2 k-tiles; its LDWEIGHTS costs ~61
# ns/pair (256 cols, no FWL), so fp8 sections are LDW-bound until the MM
# has ~140+ cols. At the K2=144 shape the fp8 MM (62.5 ns) outgrows its
# LDW entirely - pairs stream MM-bound at 62-63 ns with LDW fully hidden.
# Emission: fp8 sections GROUPED after the bf16 streams (interleaving fp8
# pairs into the bf16 stream costs ~15.5 ns extra on every following bf16
# MM - a bf16<->fp8 mode-switch cost paid per switch, 11.5 us total;
# grouped pays it ~2x per fi). Each fi's fp8 section is deferred into
# fi+1's body and its weight/x8 DMAs enqueued after fi+1's bf16 weights:
# the FIFO ring otherwise delays fi=1's startup-critical weights by the
# 0.73MB of fp8 bytes (was a 2.2-2.7 us PE gap at ~12 us).
# Stream accounting at C1=392: 1536 bf16 MMs x 165.8 + 768 fp8 pairs x
# 62.5 + 42 warmup x 109 + ~45 switches x 15 = 309.5 us busy, 0.4 us gaps,
# 7.1 us fixed runtime preamble, 5.1 us tail (last store chain ~2.5 us +
# completion barrier ~2.6 us; a split last chunk measured no better).
# Warmup 42 dummies is tuned: 24 measured +2.0 us (cold-clock exposure),
# 38 measured equal within the +-0.8 us run noise.
#
# Rejected with measurements: fp8-everything (2x PE, ~185 us, but 6.55%
# err fails the gate); threshold-based tiers (per-tier capacity = max over
# cores reintroduces padding; rank-based is pad-free); hi/lo residual
# splitting (xh@wh + xl@wh + xh@wl needs 3 DR column-passes vs 2
# bf16-equivalents - DR doubles contraction per pass, not column rate, so
# it is 1.5x SLOWER than bf16; one-sided splits of the LDW-bound sections
# double the fp8 MM time and erase the tier's entire win); expert
# pairing/F-splits (SPMD-uniform capacities eat the balance win: CA+CB >=
# 536+501); int8/uint8 matmul (not a TRN2 matmul dtype); e3m4 (4 mantissa
# bits but no DoubleRow -> 1x only); MX/x4 formats (trn3-only).
# fp8 DoublePixel/DoubleColumn are unexplored (cost model: "TODO").
#
# --- bf16 baseline architecture (unchanged) ---
# Device layout is fully weight-stationary, tokens always moving in columns:
#   g[f,t] = sum_d wg[d,f] x[d,t]   (lhsT = 128x128 wg tile, rhs = xT cols)
#   u[f,t] likewise; h[f,t] = silu(g)*u lands directly in [f,t] layout, so
#   the down matmul y[d,t] = sum_f wd[f,d] h[f,t] needs NO PE transposes.
# Router pre-pass on host picks top-2 (selection gap >> fp32 noise),
# computes renormalized weights in float64, gathers each expert's tokens
# into transposed capacity buffers, zero-pads tails (MLP(0)=0, wb=0).
# Weights host-retiled so every weight DMA is one [128, D|F] contiguous
# block. ~5us of throwaway matmuls at program start keep the PE busy while
# startup DMAs land so the HAM clock-gate opens to 8/8 before the stream.
# Startup DMA order on the sync HWDGE ring: fi=0 gate weights, first x
# group, fi=0 up weights, remaining x groups; fi=0 interleaves gate/up per
# 4-d-tile group so the PE chases the incoming x stream. Output is yT; the
# host scatter-adds transposes into [T, D].

import os
import numpy as np
import ml_dtypes

_BF16NP = ml_dtypes.bfloat16
_E4NP = ml_dtypes.float8_e4m3   # == TRN FP8_EXP4 (bias 7, max 240)

import concourse.bacc as bacc
import concourse.mybir as mybir
import concourse.tile as tile
from concourse import bass_utils

FP32 = mybir.dt.float32
BF16 = mybir.dt.bfloat16
FP8 = mybir.dt.float8e4
ACTF = mybir.ActivationFunctionType
PM = mybir.MatmulPerfMode

T, D, F, E = 2048, 2048, 4096, 8
NCORES = 8
ND = D // 128    # 16 d-tiles
NF = F // 128    # 32 f-tiles

K2 = 144         # fp8-tier capacity (multiple of 16); per-expert count is
                 # k2_e = load_e - C1 so the bf16 tier is exactly full for
                 # every expert and all padding lands in the fp8 tier, where
                 # zero columns ride (nearly) free under the LDWEIGHTS roof
SG = 32.0        # wg8 pre-scale (folded out via silu's scale input)
SU = 8.0         # wu8 pre-scale (rides through h8; folded into wb8)
SD = 32.0        # wd8 pre-scale (folded into wb8)
SWI = True       # DoubleRowSwInterleave weight layout


def _chunks_for(C):
    """Split C token columns into PSUM-bank-sized chunks (<=512 fp32 cols)."""
    nch = (C + 511) // 512
    out, rem, c0 = [], C, 0
    for i in range(nch):
        cn = -(-(rem // (nch - i)) // 4) * 4
        cn = min(cn, rem)
        out.append((c0, cn))
        c0 += cn
        rem -= cn
    return out


def build_program(C1):
    chunks = _chunks_for(C1)
    pm = PM.DoubleRowSwInterleave if SWI else PM.DoubleRow
    nc = bacc.Bacc(
        "TRN2",
        target_bir_lowering=False,
        debug=False,
        enable_asserts=False,
        num_devices=NCORES,
    )
    # bf16-tier x in [p, d, t] tile layout [128, ND*C1]
    x_d = nc.dram_tensor("x", [128, ND * C1], BF16, kind="ExternalInput").ap()
    # fp8-tier x, same layout with C=K2
    x8_d = nc.dram_tensor("x8", [128, ND * K2], FP8, kind="ExternalInput").ap()
    # router weight per token, broadcast to [128, C] on host, fp32
    wb_d = nc.dram_tensor("wb", [128, C1], FP32, kind="ExternalInput").ap()
    wb8_d = nc.dram_tensor("wb8", [128, K2], FP32, kind="ExternalInput").ap()
    # retiled weights: wg/wu rows fi*128+p, cols d*128+q (= wg[d*128+p, fi*128+q])
    wg_d = nc.dram_tensor("wg", [F, D], BF16, kind="ExternalInput").ap()
    wu_d = nc.dram_tensor("wu", [F, D], BF16, kind="ExternalInput").ap()
    # retiled wd: rows dt*128+p, cols fi*128+q  (= wd[fi*128+p, dt*128+q])
    wd_d = nc.dram_tensor("wd", [D, F], BF16, kind="ExternalInput").ap()
    # fp8 copies (pre-scaled, SwInterleaved pair layout)
    wg8_d = nc.dram_tensor("wg8", [F, D], FP8, kind="ExternalInput").ap()
    wu8_d = nc.dram_tensor("wu8", [F, D], FP8, kind="ExternalInput").ap()
    wd8_d = nc.dram_tensor("wd8", [D, F], FP8, kind="ExternalInput").ap()
    # outputs yT fp32, per tier
    y_d = nc.dram_tensor("y", [D, C1], FP32, kind="ExternalOutput").ap()
    y8_d = nc.dram_tensor("y8", [D, K2], FP32, kind="ExternalOutput").ap()

    with tile.TileContext(nc) as tc:
        with (
            tc.tile_pool(name="const", bufs=1) as const_pool,
            tc.tile_pool(name="xp", bufs=1) as x_pool,
            tc.tile_pool(name="hp", bufs=1) as h_pool,
            tc.tile_pool(name="wgu", bufs=8) as wgu_pool,
            tc.tile_pool(name="wgu8", bufs=8) as wgu8_pool,
            tc.tile_pool(name="wdp", bufs=3) as wd_pool,
            tc.tile_pool(name="wdp8", bufs=3) as wd8_pool,
            tc.tile_pool(name="yp", bufs=4) as y_pool,
            tc.tile_pool(name="stp", bufs=4) as st_pool,
            tc.tile_pool(name="ps", bufs=4, space="PSUM") as ps_pool,
            tc.tile_pool(name="ps8", bufs=4, space="PSUM") as ps8_pool,
        ):
            # ---- PE warmup: throwaway matmuls so the HAM clock-gate opens
            # to 8/8 while the startup DMAs land. ----
            dum = const_pool.tile([128, 160], BF16, tag="dum", name="dum")
            nc.vector.memset(dum[:], 0.0)
            pdum = ps_pool.tile([128, 512], FP32, tag="ps", name="ps")
            for _ in range(42):
                nc.tensor.matmul(pdum[:, :160], dum[:, :128], dum[:],
                                 start=True, stop=True)

            # startup-critical DMA order on the sync HWDGE ring (FIFO):
            # fi=0 bf16 gate weights, first x group, fi=0 bf16 up weights,
            # remaining x groups, fi=0 fp8 weights, fp8 x. wb deferred.
            wgt0 = wgu_pool.tile([128, D], BF16, tag="w", name="wgt")
            nc.sync.dma_start(wgt0[:], wg_d[0:128, :])
            xt = [x_pool.tile([128, 4 * C1], BF16, tag=f"x{g}", name=f"x{g}")
                  for g in range(ND // 4)]
            nc.sync.dma_start(xt[0][:], x_d[:, 0:4 * C1])
            wut0 = wgu_pool.tile([128, D], BF16, tag="w", name="wut")
            nc.sync.dma_start(wut0[:], wu_d[0:128, :])
            for g in range(1, ND // 4):
                nc.sync.dma_start(xt[g][:], x_d[:, g * 4 * C1:(g + 1) * 4 * C1])
            # fi=0 fp8 weights + x8 are enqueued inside fi=1's body, after
            # fi=1's startup-critical bf16 weights (the ring is FIFO and the
            # deferred fi=0 fp8 section doesn't run until ~18us in)
            wgt80 = wgu8_pool.tile([128, ND, 128], FP8, tag="w8", name="wgt8")
            wut80 = wgu8_pool.tile([128, ND, 128], FP8, tag="w8", name="wut8")
            x8t = x_pool.tile([128, ND, K2], FP8, tag="x8", name="x8t")

            def xs(d, c0, cn):
                return xt[d // 4][:, (d % 4) * C1 + c0:(d % 4) * C1 + c0 + cn]

            # ---- phase 1: gate/up matmuls + silu*up -> h[f, t] ----
            # The fp8 section for fi is emitted one fi later (inside fi+1's
            # body): at fi=0 this moves its weights/x8 out of the startup
            # DMA crunch; keeping the shift uniform keeps the stream simple.
            hs = h_pool.tile([128, NF, C1], BF16, tag="h", name="h")
            hs8 = h_pool.tile([128, NF, K2], FP8, tag="h8", name="h8")

            def fp8_gu_section(fi, wgt8, wut8):
                # grouped fp8 DR sections: at K2=128 the fp8 MM (~56 ns)
                # nearly covers its own LDW (~61 ns), and grouping pays the
                # bf16<->fp8 mode switch once per section instead of per pair
                pg8 = ps8_pool.tile([128, 512], FP32, tag="ps8", name="ps8")
                pu8 = ps8_pool.tile([128, 512], FP32, tag="ps8", name="ps8")
                for dp in range(ND // 2):
                    nc.tensor.matmul(
                        pg8[:, :K2], wgt8[:, 2 * dp:2 * dp + 2, :],
                        x8t[:, 2 * dp:2 * dp + 2, :],
                        start=(dp == 0), stop=(dp == ND // 2 - 1),
                        perf_mode=pm,
                    )
                for dp in range(ND // 2):
                    nc.tensor.matmul(
                        pu8[:, :K2], wut8[:, 2 * dp:2 * dp + 2, :],
                        x8t[:, 2 * dp:2 * dp + 2, :],
                        start=(dp == 0), stop=(dp == ND // 2 - 1),
                        perf_mode=pm,
                    )
                st8 = st_pool.tile([128, 512], FP32, tag="st", name="st")
                nc.scalar.activation(st8[:, :K2], pg8[:, :K2], ACTF.Silu,
                                     scale=1.0 / SG)
                nc.vector.tensor_mul(hs8[:, fi, :], st8[:, :K2], pu8[:, :K2])

            pend8 = None
            for fi in range(NF):
                if fi == 0:
                    wgt, wut, wgt8, wut8 = wgt0, wut0, wgt80, wut80
                else:
                    wgt = wgu_pool.tile([128, D], BF16, tag="w", name="wgt")
                    nc.sync.dma_start(wgt[:], wg_d[fi * 128:(fi + 1) * 128, :])
                    wut = wgu_pool.tile([128, D], BF16, tag="w", name="wut")
                    nc.sync.dma_start(wut[:], wu_d[fi * 128:(fi + 1) * 128, :])
                    if fi == 1:
                        nc.sync.dma_start(wgt80[:], wg8_d[0:128, :])
                        nc.sync.dma_start(wut80[:], wu8_d[0:128, :])
                        nc.sync.dma_start(x8t[:], x8_d[:])
                    wgt8 = wgu8_pool.tile([128, ND, 128], FP8, tag="w8",
                                          name="wgt8")
                    nc.sync.dma_start(wgt8[:], wg8_d[fi * 128:(fi + 1) * 128, :])
                    wut8 = wgu8_pool.tile([128, ND, 128], FP8, tag="w8",
                                          name="wut8")
                    nc.sync.dma_start(wut8[:], wu8_d[fi * 128:(fi + 1) * 128, :])
                pg = [ps_pool.tile([128, 512], FP32, tag="ps", name="ps")
                      for _ in chunks]
                pu = [ps_pool.tile([128, 512], FP32, tag="ps", name="ps")
                      for _ in chunks]
                if fi == 0:
                    # chase the incoming x stream per 4-d-tile group
                    for g in range(ND // 4):
                        for d in range(4 * g, 4 * g + 4):
                            lw = wgt[:, d * 128:(d + 1) * 128]
                            for ci, (c0, cn) in enumerate(chunks):
                                nc.tensor.matmul(
                                    pg[ci][:, :cn], lw, xs(d, c0, cn),
                                    start=(d == 0), stop=(d == ND - 1),
                                )
                        for d in range(4 * g, 4 * g + 4):
                            lw = wut[:, d * 128:(d + 1) * 128]
                            for ci, (c0, cn) in enumerate(chunks):
                                nc.tensor.matmul(
                                    pu[ci][:, :cn], lw, xs(d, c0, cn),
                                    start=(d == 0), stop=(d == ND - 1),
                                )
                else:
                    for d in range(ND):
                        lw = wgt[:, d * 128:(d + 1) * 128]
                        for ci, (c0, cn) in enumerate(chunks):
                            nc.tensor.matmul(
                                pg[ci][:, :cn], lw, xs(d, c0, cn),
                                start=(d == 0), stop=(d == ND - 1),
                            )
                    for d in range(ND):
                        lw = wut[:, d * 128:(d + 1) * 128]
                        for ci, (c0, cn) in enumerate(chunks):
                            nc.tensor.matmul(
                                pu[ci][:, :cn], lw, xs(d, c0, cn),
                                start=(d == 0), stop=(d == ND - 1),
                            )
                if pend8 is not None:
                    fp8_gu_section(*pend8)
                pend8 = (fi, wgt8, wut8)
                for ci, (c0, cn) in enumerate(chunks):
                    st = st_pool.tile([128, 512], FP32, tag="st", name="st")
                    nc.scalar.activation(st[:, :cn], pg[ci][:, :cn], ACTF.Silu)
                    nc.vector.tensor_mul(hs[:, fi, c0:c0 + cn], st[:, :cn],
                                         pu[ci][:, :cn])
            fp8_gu_section(*pend8)

            # wb first consumed by phase 2; keep out of the startup crunch
            wb_sb = const_pool.tile([128, C1], FP32, tag="wb", name="wb_sb")
            nc.sync.dma_start(wb_sb[:], wb_d[:])
            wb8_sb = const_pool.tile([128, K2], FP32, tag="wb8", name="wb8_sb")
            nc.sync.dma_start(wb8_sb[:], wb8_d[:])

            # ---- phase 2: down matmuls, router-weight scale, store yT ----
            for dt in range(ND):
                last = dt == ND - 1
                wdt = wd_pool.tile([128, F], BF16, tag="wd", name="wdt")
                wdt8 = wd8_pool.tile([128, NF, 128], FP8, tag="wd8",
                                     name="wdt8")
                if last:
                    # fp8 first for the last dt so its store isn't the tail
                    nc.sync.dma_start(wdt8[:], wd8_d[dt * 128:(dt + 1) * 128, :])
                    nc.sync.dma_start(wdt[:], wd_d[dt * 128:(dt + 1) * 128, :])
                else:
                    nc.sync.dma_start(wdt[:], wd_d[dt * 128:(dt + 1) * 128, :])
                    nc.sync.dma_start(wdt8[:], wd8_d[dt * 128:(dt + 1) * 128, :])

                def fp8_dn_section():
                    py8 = ps8_pool.tile([128, 512], FP32, tag="ps8",
                                        name="ps8")
                    for fp in range(NF // 2):
                        nc.tensor.matmul(
                            py8[:, :K2], wdt8[:, 2 * fp:2 * fp + 2, :],
                            hs8[:, 2 * fp:2 * fp + 2, :],
                            start=(fp == 0), stop=(fp == NF // 2 - 1),
                            perf_mode=pm,
                        )
                    y8sb = y_pool.tile([128, 512], FP32, tag="y", name="y8sb")
                    nc.vector.tensor_mul(y8sb[:, :K2], py8[:, :K2], wb8_sb[:])
                    nc.sync.dma_start(y8_d[dt * 128:(dt + 1) * 128, :],
                                      y8sb[:, :K2])

                if last:
                    fp8_dn_section()
                for ci, (c0, cn) in enumerate(chunks):
                    py = ps_pool.tile([128, 512], FP32, tag="ps", name="ps")
                    for fi in range(NF):
                        nc.tensor.matmul(
                            py[:, :cn], wdt[:, fi * 128:(fi + 1) * 128],
                            hs[:, fi, c0:c0 + cn],
                            start=(fi == 0), stop=(fi == NF - 1),
                        )
                    ysb = y_pool.tile([128, 512], FP32, tag="y", name="ysb")
                    nc.vector.tensor_mul(ysb[:, :cn], py[:, :cn],
                                         wb_sb[:, c0:c0 + cn])
                    nc.sync.dma_start(
                        y_d[dt * 128:(dt + 1) * 128, c0:c0 + cn], ysb[:, :cn])
                if not last:
                    fp8_dn_section()

    nc.compile()
    return nc


_PROGRAM_CACHE = {}


def _get_program(C1):
    if C1 not in _PROGRAM_CACHE:
        _PROGRAM_CACHE[C1] = build_program(C1)
    return _PROGRAM_CACHE[C1]


def _route_host(x_TD, router_w):
    """Host dispatch: top-2 ids + renormalized top-2 softmax weights."""
    logits = (x_TD @ router_w).astype(np.float64)  # selection gap >> fp32 err
    order = np.argsort(-logits, axis=1, kind="stable")
    top2 = order[:, :2]
    z = logits - logits.max(axis=1, keepdims=True)
    p = np.exp(z)
    p /= p.sum(axis=1, keepdims=True)
    pw = np.take_along_axis(p, top2, axis=1)       # [T, 2]
    pw /= pw.sum(axis=1, keepdims=True)
    return top2, pw


def _retile_wgu(w):
    """[D, F] -> [F, D] rows fi*128+p, cols d*128+q."""
    m = w.reshape(ND, 128, NF, 128).transpose(2, 1, 0, 3)
    return np.ascontiguousarray(m).reshape(F, D)


def _retile_wd(w):
    """[F, D] -> [D, F] rows dt*128+p, cols fi*128+q."""
    m = w.reshape(NF, 128, ND, 128).transpose(2, 1, 0, 3)
    return np.ascontiguousarray(m).reshape(D, F)


def _swinterleave(w):
    """[128, n, 128] retiled rows -> DoubleRowSwInterleave pair layout.

    Physical per-partition pair order: A127,B127,A126,B126,...,A0,B0."""
    p, n, m = w.shape
    a = w[:, 0::2, ::-1]    # [p, n/2, m] reversed cols
    b = w[:, 1::2, ::-1]
    il = np.empty((p, n // 2, 2 * m), w.dtype)
    il[:, :, 0::2] = a
    il[:, :, 1::2] = b
    return il.reshape(p, n, m)


def _fp8_weights(w, scale, down):
    """Quantize a [D,F]/[F,D] weight to scaled e4m3 in retiled+DR layout."""
    q = (w * scale).astype(_E4NP)
    r = _retile_wd(q) if down else _retile_wgu(q)   # [nrow, ncol]
    nrow, ncol = r.shape
    r3 = r.reshape(nrow // 128, 128, ncol // 128, 128)
    if SWI:
        r3 = np.stack([_swinterleave(blk) for blk in r3])
    return np.ascontiguousarray(r3.reshape(nrow, ncol))


def _xe_tile(xg, Ccap, dtype):
    """[n<=Ccap, D] tokens -> [128, ND*Ccap] [p, d, t] tile layout."""
    n = xg.shape[0]
    buf = np.zeros((Ccap, D), np.float32)
    buf[:n] = xg
    m = buf.T.astype(dtype).reshape(ND, 128, Ccap).transpose(1, 0, 2)
    return np.ascontiguousarray(m).reshape(128, ND * Ccap)


def kernel_with_results(x_TD, router_w, w_gate, w_up, w_down):
    x_TD = np.ascontiguousarray(x_TD, np.float32)
    router_w = np.ascontiguousarray(router_w, np.float32)
    w_gate = np.ascontiguousarray(w_gate, np.float32)
    w_up = np.ascontiguousarray(w_up, np.float32)
    w_down = np.ascontiguousarray(w_down, np.float32)

    top2, pw = _route_host(x_TD, router_w)
    # per expert: slot lists (token idx, weight), sorted by weight ascending
    slot_ix, slot_w = [], []
    for e in range(E):
        ixs, ws = [], []
        for s in range(2):
            ix = np.where(top2[:, s] == e)[0]
            ixs.append(ix)
            ws.append(pw[ix, s])
        allix = np.concatenate(ixs)
        allw = np.concatenate(ws)
        o = np.argsort(allw, kind="stable")
        slot_ix.append(allix[o])
        slot_w.append(allw[o])
    loads = [len(ix) for ix in slot_ix]
    C1 = max(64, -(-(max(loads) - K2) // 8) * 8)
    k2s = [max(0, min(K2, L - C1)) for L in loads]

    nc = _get_program(C1)

    in_maps = []
    for e in range(E):
        k2 = k2s[e]
        ix8 = slot_ix[e][:k2]
        w8 = slot_w[e][:k2]
        ix1 = slot_ix[e][k2:]
        w1 = slot_w[e][k2:]
        n1 = len(ix1)

        wtok = np.zeros((C1,), np.float32)
        wtok[:n1] = w1
        wb = np.ascontiguousarray(
            np.broadcast_to(wtok[None, :], (128, C1)), np.float32)
        wtok8 = np.zeros((K2,), np.float32)
        wtok8[:k2] = w8 / (SU * SD)
        wb8 = np.ascontiguousarray(
            np.broadcast_to(wtok8[None, :], (128, K2)), np.float32)

        x8buf = np.zeros((K2, D), np.float32)
        x8buf[:k2] = x_TD[ix8]

        in_maps.append({
            "x": _xe_tile(x_TD[ix1], C1, _BF16NP),
            "x8": _xe_tile(x8buf, K2, _E4NP),
            "wb": wb,
            "wb8": wb8,
            "wg": _retile_wgu(w_gate[e].astype(_BF16NP)),
            "wu": _retile_wgu(w_up[e].astype(_BF16NP)),
            "wd": _retile_wd(w_down[e].astype(_BF16NP)),
            "wg8": _fp8_weights(w_gate[e], SG, False),
            "wu8": _fp8_weights(w_up[e], SU, False),
            "wd8": _fp8_weights(w_down[e], SD, True),
        })

    try:
        res = bass_utils.run_bass_kernel_spmd(
            nc, in_maps, core_ids=list(range(NCORES))
        )
    except ModuleNotFoundError:
        os.environ["BASS_NEVER_TRACE"] = "1"
        res = bass_utils.run_bass_kernel_spmd(
            nc, in_maps, core_ids=list(range(NCORES))
        )

    out = np.zeros((T, D), np.float32)
    for e in range(E):
        k2 = k2s[e]
        ix8 = slot_ix[e][:k2]
        ix1 = slot_ix[e][k2:]
        y = res.results[e]["y"]    # [D, C1]
        y8 = res.results[e]["y8"]  # [D, K2]
        out[ix1] += y[:, :len(ix1)].T
        out[ix8] += y8[:, :len(ix8)].T
    return out, res


def kernel(**inputs):
    out, _ = kernel_with_results(**inputs)
    return out
